# revision 9
# baseline (speedup 1.0000x reference)
"""Trainium2 Bass kernel for nn_Crude_Diag: y = x @ W.T with W strictly diagonal.

Since W is diagonal, y[i, j] = x[i, j] * diag(W)[j] — a memory-bound
column-wise scale. Strategy (per sharding hint): data-parallel over the token
dim across 8 NeuronCores; the length-n diagonal is replicated to every core.

Per core: the [1024, 4096] f32 shard streams through SBUF in [128, 4096]
tiles. Every load/store is split in half across both HWDGE rings (SP + ACT)
— a single ring tops out around ~215 GB/s, both together sustain the
~435 GB/s SBUF fabric ceiling. The diagonal is shipped as a 16 KiB [1, 4096]
row and broadcast across the 128 partitions on-chip with a ones-matmul on
the (otherwise idle) tensor engine, which is bit-exact for f32.
"""

import numpy as np

import concourse.bacc as bacc
import concourse.mybir as mybir
import concourse.tile as tile
from concourse.bass_utils import run_bass_kernel_spmd

TOKENS = 8192
FEATS = 4096
NCORES = 8
ROWS = TOKENS // NCORES  # rows per core
P = 128  # SBUF partitions
H = FEATS // 2  # half the free dim: one half per HWDGE ring

# test.py can flip these to capture an NTFF profile of the run.
PROFILE = False
LAST_RESULTS = None

_nc_cache = None


def _build_bass():
    """Build + compile the per-core Bass module (cached across calls)."""
    global _nc_cache
    if _nc_cache is not None:
        return _nc_cache

    nc = bacc.Bacc("TRN2", target_bir_lowering=False, debug=False)
    x = nc.dram_tensor("x", [ROWS, FEATS], mybir.dt.float32, kind="ExternalInput")
    d = nc.dram_tensor("d", [1, FEATS], mybir.dt.float32, kind="ExternalInput")
    y = nc.dram_tensor("y", [ROWS, FEATS], mybir.dt.float32, kind="ExternalOutput")

    NT = ROWS // P
    # Half-tile (1 MiB) DMAs spread round-robin over the three available DMA
    # queues: gpsimd (SWDGE q0), sync (HWDGE q1), scalar (HWDGE q10). Any
    # single queue tops out near ~215 GB/s; three together keep the
    # HBM/fabric limit saturated and fill each other's completion gaps.
    engines = ["gpsimd", "sync", "scalar"]
    with tile.TileContext(nc) as tc:
        with (
            tc.tile_pool(name="const", bufs=1) as cpool,
            tc.tile_pool(name="psum", bufs=1, space="PSUM") as ppool,
            tc.tile_pool(name="io", bufs=NT) as pool,
        ):
            # Ship the diagonal as one 16 KiB row; broadcast it across the
            # 128 partitions with ones[128,1] @ diag[1,512] per PSUM bank on
            # the otherwise-idle tensor engine (bit-exact for f32). The
            # multiplies read it straight out of PSUM.
            diag_row = cpool.tile([1, FEATS], mybir.dt.float32)
            nc.sync.dma_start(out=diag_row[:], in_=d[:])
            ones = cpool.tile([1, P], mybir.dt.float32)
            nc.vector.memset(ones[:], 1.0)
            pd = ppool.tile([P, FEATS], mybir.dt.float32)
            for j in range(FEATS // 512):
                nc.tensor.matmul(
                    pd[:, j * 512:(j + 1) * 512], ones[:],
                    diag_row[:, j * 512:(j + 1) * 512], start=True, stop=True,
                )

            # All loads first (one SBUF slot per tile), then the multiplies,
            # then all stores — so late loads are never queued behind stores.
            k = 0
            tiles = []
            for i in range(NT):
                t = pool.tile([P, FEATS], mybir.dt.float32)
                rs = slice(i * P, (i + 1) * P)
                for h in range(2):
                    cs = slice(h * H, (h + 1) * H)
                    getattr(nc, engines[k % 3]).dma_start(out=t[:, cs], in_=x[rs, cs])
                    k += 1
                tiles.append(t)
            for t in tiles:
                nc.vector.tensor_mul(out=t[:], in0=t[:], in1=pd[:])
            for i, t in enumerate(tiles):
                rs = slice(i * P, (i + 1) * P)
                for h in range(2):
                    cs = slice(h * H, (h + 1) * H)
                    getattr(nc, engines[k % 3]).dma_start(out=y[rs, cs], in_=t[:, cs])
                    k += 1

    nc.compile()
    _nc_cache = nc
    return nc


def kernel(x: np.ndarray, W: np.ndarray) -> np.ndarray:
    global LAST_RESULTS
    x = np.ascontiguousarray(np.asarray(x, dtype=np.float32))
    W = np.asarray(W, dtype=np.float32)
    assert x.shape == (TOKENS, FEATS), x.shape

    # y = x @ W.T with diagonal W collapses to scaling column j by W[j, j].
    diag = np.ascontiguousarray(np.diagonal(W)).astype(np.float32).reshape(1, FEATS)

    nc = _build_bass()
    in_maps = [
        {"x": x[c * ROWS:(c + 1) * ROWS], "d": diag} for c in range(NCORES)
    ]
    res = run_bass_kernel_spmd(
        nc, in_maps, core_ids=list(range(NCORES)), trace=PROFILE
    )
    LAST_RESULTS = res
    return np.concatenate([r["y"] for r in res.results], axis=0)
